# revision 1
# baseline (speedup 1.0000x reference)
import sys as _sys

for _p in ("/opt/trn_rl_repo",):
    if _p not in _sys.path:
        _sys.path.insert(0, _p)

import os
from contextlib import ExitStack

import numpy as np
import ml_dtypes

import concourse.bass as bass
import concourse.mybir as mybir
from concourse.bass_utils import run_bass_kernel_spmd

bf16 = ml_dtypes.bfloat16
F32 = mybir.dt.float32
BF = mybir.dt.bfloat16

B, V, LAT, U, P = 256, 42, 200, 512, 3
T = int(os.environ.get("KERNEL_T", "128"))
NCORES = 8
BL = B // NCORES
G4 = 4 * U
KT = U // 128
AF = mybir.ActivationFunctionType
OP = mybir.AluOpType

XC_BUFS = 4
NZB = 3


class Tracker:

    def __init__(self):
        self.vals = {}
        self.counts = {}

    def inc(self, sem, tag, n=1):
        c = self.counts.get(sem, 0) + n
        self.counts[sem] = c
        key = (sem, tag)
        assert key not in self.vals, f"dup inc tag {key}"
        self.vals[key] = c
        return c

    def val(self, sem, tag):
        return self.vals[(sem, tag)]


def _build():
    nc = bass.Bass()

    dp = lambda n, s, d, o=False: nc.declare_dram_parameter(n, list(s), d, isOutput=o)
    xc_enc_d = dp("xc_enc", [T, 128, 512], BF)
    xc_dec_d = dp("xc_dec", [T, 128, 512], BF)
    rk0e_d = dp("rk0e", [128, KT, G4], BF)
    w1e_d = dp("w1e", [128, 2 * KT, G4], BF)
    w2e_d = dp("w2e", [128, 2 * KT, G4], BF)
    rk0d_d = dp("rk0d", [128, KT, G4], BF)
    w1d_d = dp("w1d", [128, 2 * KT, G4], BF)
    w2d_d = dp("w2d", [128, 2 * KT, G4], BF)
    dk0z_d = dp("dk0z", [128, 2, G4], BF)
    wm_d = dp("wm", [128, KT, LAT], BF)
    ws_d = dp("ws", [128, KT, LAT], BF)
    wo_d = dp("wo", [128, KT, V], BF)
    ohm_d = dp("ohm", [BL, T * V], BF)
    maskf_d = dp("maskf", [BL, T], F32)
    eps_d = dp("eps", [BL, LAT], F32)
    iden_d = dp("iden", [128, 128], F32)
    idenb_d = dp("idenb", [128, 128], BF)
    ones32_d = dp("ones32", [BL, 1], F32)
    out_d = dp("out", [1, 4], F32, o=True)

    tk = Tracker()

    with ExitStack() as ctx:
        _nn = [0]

        def sbt(*s):
            _nn[0] += 1
            return ctx.enter_context(
                nc.sbuf_tensor(f"sb{_nn[0]}", list(s[:-1]), s[-1]))

        def pst(*s):
            _nn[0] += 1
            return ctx.enter_context(
                nc.psum_tensor(f"ps{_nn[0]}", list(s[:-1]), s[-1]))

        rk0_sb = sbt(128, KT * G4, BF)
        w1_sb = sbt(128, 2 * KT * G4, BF)
        w2_sb = sbt(128, 2 * KT * G4, BF)
        dk0z_sb = sbt(128, 2 * G4, BF)
        wm_sb = sbt(128, KT * LAT, BF)
        ws_sb = sbt(128, KT * LAT, BF)
        wo_sb = sbt(128, KT * V, BF)
        iden_sb = sbt(128, 128, F32)
        idenb_sb = sbt(128, 128, BF)
        ones32_sb = sbt(BL, 1, F32)
        xc_sb = [sbt(128, 512, BF) for _ in range(XC_BUFS)]
        hT = [sbt(128, KT * BL, BF) for _ in range(3)]
        cT_sb = sbt(128, KT * BL, BF)
        zrT_sb = sbt(128, 2 * BL, BF)
        sg = [sbt(128, 512, F32) for _ in range(3)]
        t1b = [sbt(128, 128, F32) for _ in range(3)]
        t2b = [sbt(128, 128, F32) for _ in range(3)]
        tcb = [sbt(128, 128, F32) for _ in range(3)]
        h2c = [sbt(128, 128, BF) for _ in range(3)]
        cst = [sbt(128, 128, F32) for _ in range(3)]
        zpart_sb = sbt(128, 512, F32)
        ohm_sb = sbt(BL, T * V, BF)
        maskf_sb = sbt(BL, T, F32)
        eps_sb = sbt(BL, LAT, F32)
        mean_sb = sbt(BL, LAT, F32)
        ls_sb = sbt(BL, LAT, F32)
        els2_sb = sbt(BL, LAT, F32)
        zr_sb = sbt(BL, LAT, F32)
        pre_buf = sbt(BL, T * V, F32)
        se_sb = sbt(BL, T, F32)
        m2_sb = sbt(BL, LAT, F32)
        els_sb = sbt(BL, LAT, F32)
        partial_sb = sbt(BL, 4, F32)
        out_sb = sbt(1, 4, F32)

        zp = [pst(128, 512, F32) for _ in range(NZB)]
        trC = [pst(128, 128, BF), pst(128, 128, BF)]
        preP = [pst(BL, 512, F32), pst(BL, 512, F32)]
        trF = pst(128, 128, F32)

        s_pe = ctx.enter_context(nc.semaphore("s_pe"))
        s_act = ctx.enter_context(nc.semaphore("s_act"))
        s_dve = ctx.enter_context(nc.semaphore("s_dve"))
        s_gp = ctx.enter_context(nc.semaphore("s_gp"))
        s_w = ctx.enter_context(nc.semaphore("s_w"))
        s_out = ctx.enter_context(nc.semaphore("s_out"))
        s_xcb = [ctx.enter_context(nc.semaphore(f"s_xc{i}"))
                 for i in range(XC_BUFS)]
        blk = ctx.enter_context(nc.Block())

        SEMS = {"pe": s_pe, "act": s_act, "dve": s_dve, "gp": s_gp,
                "w": s_w, "out": s_out}
        for i in range(XC_BUFS):
            SEMS[f"xc{i}"] = s_xcb[i]

        class Em:
            def __init__(self, real, eng=None):
                self.real = real
                self.eng = eng

            def wait(self, sem, tag):
                if self.real:
                    self.eng.wait_ge(SEMS[sem], tk.val(sem, tag))

            def wait_val(self, sem, v):
                if self.real:
                    self.eng.wait_ge(SEMS[sem], v)

            def inc(self, ins, sem, tag, n=1):
                if not self.real:
                    tk.inc(sem, tag, n)
                else:
                    ins.then_inc(SEMS[sem], n)

        zburst_list = []
        for ph in ("enc", "dec"):
            if ph == "dec":
                zburst_list.append(("zpart",))
            for s in range(T + 3):
                for l, t in ((0, s), (1, s - 1), (2, s - 2)):
                    if 0 <= t < T:
                        zburst_list.append(("z", ph, l, t))
        zb_index = {e: i for i, e in enumerate(zburst_list)}

        def zburst_reader(e):
            if e[0] == "zpart":
                return ("dve", ("p2_zpart_copy",))
            _, ph, l, t = e
            return ("act", ("sg", ph, l, t))

        tr_list = []
        for ph in ("enc", "dec"):
            for s in range(T + 3):
                for (l, t) in ((2, s - 3), (0, s), (1, s - 1)):
                    if 0 <= t < T:
                        tr_list.append(("tr", ph, l, t))
        tr_index = {e: i for i, e in enumerate(tr_list)}

        def tr_reader(e):
            _, ph, l, t = e
            return ("dve", ("cp", ph, l, t))

        def wsb_for(l):
            return (rk0_sb, w1_sb, w2_sb)[l]

        def pe_zburst(em, ph, l, t):
            e = ("z", ph, l, t)
            zi = zb_index[e]
            bank = zp[zi % NZB]
            if zi >= NZB:
                em.wait(*zburst_reader(zburst_list[zi - NZB]))
            if t == 0:
                if ph == "enc":
                    em.wait("w", ("w_rk0e",) if l == 0 else
                            (("w_w1e",) if l == 1 else ("w_w2e",)))
                else:
                    em.wait("w", ("w_dec",))
            if t == 0:
                em.wait("dve", ("init",) if ph == "enc" else ("init2",))
            else:
                em.wait("dve", ("cp", ph, l, t - 1))
            if l > 0:
                em.wait("dve", ("cp", ph, l - 1, t))
            wsb = wsb_for(l)
            nk = KT if l == 0 else 2 * KT

            def stat(k):
                if l == 0 or k < KT:
                    src = hT[l - 1] if l > 0 else hT[0]
                    return src[:, k * BL:(k + 1) * BL]
                return hT[l][:, (k - KT) * BL:(k - KT + 1) * BL]

            if l == 0:
                def stat(k):
                    return hT[0][:, k * BL:(k + 1) * BL]
            else:
                def stat(k, l=l):
                    if k < KT:
                        return hT[l - 1][:, k * BL:(k + 1) * BL]
                    return hT[l][:, (k - KT) * BL:(k - KT + 1) * BL]

            if em.real:
                for k in range(nk):
                    for g in range(4):
                        ins = nc.tensor.matmul(
                            bank[32 * g:32 * (g + 1), :],
                            stat(k),
                            wsb[:, (k * 4 + g) * 512:(k * 4 + g + 1) * 512],
                            start=(k == 0), stop=(k == nk - 1),
                            tile_position=(0, 32 * g),
                            skip_group_check=True,
                        )
                em.inc(ins, "pe", e)
            else:
                em.inc(None, "pe", e)

        def pe_tr(em, ph, l, t):
            e = ("tr", ph, l, t)
            ti = tr_index[e]
            dst = trC[ti % 2]
            if ti >= 2:
                em.wait(*tr_reader(tr_list[ti - 2]))
            em.wait("dve", ("h2", ph, l, t))
            if em.real:
                ins = nc.tensor.transpose(dst[:], h2c[l][:], idenb_sb[:])
                em.inc(ins, "pe", e)
            else:
                em.inc(None, "pe", e)

        def pe_proj(em, t):
            em.wait("dve", ("cp", "dec", 2, t))
            if t >= 2:
                em.wait("act", ("precopy", t - 2))
            elif t == 0:
                em.wait("act", ("p2_mean_sb",))
            else:
                em.wait("act", ("p2_ls_sb",))
            pp = preP[t % 2]
            if em.real:
                for k in range(KT):
                    ins = nc.tensor.matmul(
                        pp[:, 0:V], hT[2][:, k * BL:(k + 1) * BL],
                        wo_sb[:, k * V:(k + 1) * V],
                        start=(k == 0), stop=(k == KT - 1))
                em.inc(ins, "pe", ("proj", t))
            else:
                em.inc(None, "pe", ("proj", t))

        def pe_body(e, real):
            em = Em(real, e)
            for ph in ("enc", "dec"):
                if ph == "dec":
                    em.wait("dve", ("c2", "enc", 2, T - 1))
                    if em.real:
                        ins = nc.tensor.transpose(trF[:], cst[2][:],
                                                  iden_sb[:])
                        em.inc(ins, "pe", ("trCT",))
                    else:
                        em.inc(None, "pe", ("trCT",))
                    em.wait("dve", ("p2_cT",))
                    for dst, wsb2, tg in ((preP[0], wm_sb, "p2_mm_mean"),
                                          (preP[1], ws_sb, "p2_mm_ls")):
                        if em.real:
                            for k in range(KT):
                                ins = nc.tensor.matmul(
                                    dst[:, 0:LAT],
                                    cT_sb[:, k * BL:(k + 1) * BL],
                                    wsb2[:, k * LAT:(k + 1) * LAT],
                                    start=(k == 0), stop=(k == KT - 1))
                            em.inc(ins, "pe", (tg,))
                        else:
                            em.inc(None, "pe", (tg,))
                    em.wait("dve", ("p2_zr",))
                    em.wait("dve", ("p2_cT",))
                    dstz = trF
                    if em.real:
                        nc.tensor.transpose(dstz[:, 0:BL], zr_sb[:, 0:128],
                                            iden_sb[0:BL, 0:BL])
                        ins = nc.tensor.transpose(
                            dstz[0:LAT - 128, BL:2 * BL], zr_sb[:, 128:LAT],
                            iden_sb[0:BL, 0:BL])
                        em.inc(ins, "pe", ("trZR",))
                    else:
                        em.inc(None, "pe", ("trZR",))
                    em.wait("dve", ("p2_zrT",))
                    zi = zb_index[("zpart",)]
                    em.wait(*zburst_reader(zburst_list[zi - NZB]))
                    bank = zp[zi % NZB]
                    if em.real:
                        for k in range(2):
                            for g in range(4):
                                ins = nc.tensor.matmul(
                                    bank[32 * g:32 * (g + 1), :],
                                    zrT_sb[:, k * BL:(k + 1) * BL],
                                    dk0z_sb[:, (k * 4 + g) * 512:
                                            (k * 4 + g + 1) * 512],
                                    start=(k == 0), stop=(k == 1),
                                    tile_position=(0, 32 * g),
                                    skip_group_check=True,
                                )
                        em.inc(ins, "pe", ("zpart",))
                    else:
                        em.inc(None, "pe", ("zpart",))

                for s in range(T + 3):
                    if ph == "dec" and 0 <= s - 3 < T:
                        t2_ = s - 3
                        if ("tr", ph, 2, t2_) in tr_index:
                            pe_tr(em, ph, 2, t2_)
                    elif 0 <= s - 3 < T:
                        pe_tr(em, ph, 2, s - 3)
                    if s < T:
                        pe_zburst(em, ph, 0, s)
                    if 0 <= s - 1 < T:
                        pe_zburst(em, ph, 1, s - 1)
                    if ph == "dec" and 0 <= s - 3 < T:
                        pe_proj(em, s - 3)
                    if 0 <= s - 2 < T:
                        pe_zburst(em, ph, 2, s - 2)
                    if s < T:
                        pe_tr(em, ph, 0, s)
                    if 0 <= s - 1 < T:
                        pe_tr(em, ph, 1, s - 1)

            em.wait("dve", ("f_kl",))
            if em.real:
                ins = nc.tensor.matmul(preP[0][0:1, 300:304], ones32_sb[:],
                                       partial_sb[:], start=True, stop=True)
                em.inc(ins, "pe", ("f_red",))
            else:
                em.inc(None, "pe", ("f_red",))

        def act_sig(em, ph, l, t):
            if l == 0:
                em.wait("dve", ("xcz", ph, t))
            else:
                em.wait("pe", ("z", ph, l, t))
            if t > 0:
                em.wait("dve", ("h2", ph, l, t - 1))
                em.wait("gp", ("t2", ph, l, t - 1))
            elif ph == "dec":
                pe_ = ("h2", "enc", l, T - 1)
                em.wait("dve", pe_)
                em.wait("gp", ("t2", "enc", l, T - 1))
            zi = zb_index[("z", ph, l, t)]
            bank = zp[zi % NZB]
            if em.real:
                nc.scalar.activation(sg[l][:, 0:384], bank[:, 0:384],
                                     AF.Sigmoid)
                ins = nc.scalar.activation(sg[l][:, 384:512],
                                           bank[:, 384:512], AF.Tanh)
                em.inc(ins, "act", ("sg", ph, l, t))
            else:
                em.inc(None, "act", ("sg", ph, l, t))

        def act_tc(em, ph, l, t):
            em.wait("dve", ("c2", ph, l, t))
            if em.real:
                ins = nc.scalar.activation(tcb[l][:], cst[l][:], AF.Tanh)
                em.inc(ins, "act", ("tc", ph, l, t))
            else:
                em.inc(None, "act", ("tc", ph, l, t))

        def act_body(e, real):
            em = Em(real, e)
            for ph in ("enc", "dec"):
                if ph == "dec":
                    em.wait("pe", ("p2_mm_mean",))
                    if em.real:
                        ins = nc.scalar.copy(mean_sb[:], preP[0][:, 0:LAT])
                        em.inc(ins, "act", ("p2_mean_sb",))
                    else:
                        em.inc(None, "act", ("p2_mean_sb",))
                    em.wait("pe", ("p2_mm_ls",))
                    if em.real:
                        ins = nc.scalar.copy(ls_sb[:], preP[1][:, 0:LAT])
                        em.inc(ins, "act", ("p2_ls_sb",))
                    else:
                        em.inc(None, "act", ("p2_ls_sb",))
                    if em.real:
                        ins = nc.scalar.activation(els2_sb[:], ls_sb[:],
                                                   AF.Exp, scale=0.5)
                        em.inc(ins, "act", ("p2_exp",))
                    else:
                        em.inc(None, "act", ("p2_exp",))
                for s in range(T + 3):
                    for l, t in ((0, s), (1, s - 1), (2, s - 2)):
                        if not (0 <= t < T):
                            continue
                        act_sig(em, ph, l, t)
                        act_tc(em, ph, l, t)
                    if ph == "dec" and 0 <= s - 3 < T:
                        tt = s - 3
                        em.wait("pe", ("proj", tt))
                        if em.real:
                            ins = nc.scalar.copy(
                                pre_buf[:, tt * V:(tt + 1) * V],
                                preP[tt % 2][:, 0:V])
                            em.inc(ins, "act", ("precopy", tt))
                        else:
                            em.inc(None, "act", ("precopy", tt))
            em.wait("dve", ("f_picked",))
            if em.real:
                ins = nc.scalar.activation(pre_buf[:], pre_buf[:], AF.Exp)
                em.inc(ins, "act", ("f_exp",))
            else:
                em.inc(None, "act", ("f_exp",))
            em.wait("dve", ("f_se",))
            if em.real:
                ins = nc.scalar.activation(se_sb[:], se_sb[:], AF.Ln)
                em.inc(ins, "act", ("f_ln",))
            else:
                em.inc(None, "act", ("f_ln",))
            if em.real:
                nc.scalar.activation(m2_sb[:], mean_sb[:], AF.Square)
                ins = nc.scalar.activation(els_sb[:], ls_sb[:], AF.Exp)
                em.inc(ins, "act", ("f_m2els",))
            else:
                em.inc(None, "act", ("f_m2els",))
            em.wait("pe", ("f_red",))
            if em.real:
                ins = nc.scalar.copy(out_sb[:], preP[0][0:1, 300:304])
                em.inc(ins, "act", ("f_out",))
            else:
                em.inc(None, "act", ("f_out",))

        def dve_cell(em, ph, l, t):
            em.wait("act", ("sg", ph, l, t))
            if em.real:
                nc.vector.tensor_tensor(t1b[l][:], sg[l][:, 128:256],
                                        cst[l][:], OP.mult)
            em.wait("gp", ("t2", ph, l, t))
            if t > 0:
                em.wait("act", ("tc", ph, l, t - 1))
            elif ph == "dec":
                em.wait("act", ("tc", "enc", l, T - 1))
            if em.real:
                ins = nc.vector.tensor_tensor(cst[l][:], t1b[l][:],
                                              t2b[l][:], OP.add)
                em.inc(ins, "dve", ("c2", ph, l, t))
            else:
                em.inc(None, "dve", ("c2", ph, l, t))
            em.wait("act", ("tc", ph, l, t))
            if t > 0:
                em.wait("pe", ("tr", ph, l, t - 1))
            elif ph == "dec" and ("tr", "enc", l, T - 1) in tr_index:
                em.wait("pe", ("tr", "enc", l, T - 1))
            if em.real:
                ins = nc.vector.tensor_tensor(h2c[l][:], sg[l][:, 256:384],
                                              tcb[l][:], OP.mult)
                em.inc(ins, "dve", ("h2", ph, l, t))
            else:
                em.inc(None, "dve", ("h2", ph, l, t))

        def dve_copy(em, ph, l, t):
            em.wait("pe", ("tr", ph, l, t))
            ti = tr_index[("tr", ph, l, t)]
            src = trC[ti % 2]
            if em.real:
                ins = nc.vector.tensor_copy(hT[l][:], src[:])
                em.inc(ins, "dve", ("cp", ph, l, t))
            else:
                em.inc(None, "dve", ("cp", ph, l, t))

        def dve_xcadd(em, ph, t, xcount):
            em.wait_val(f"xc{xcount % XC_BUFS}",
                        16 * (xcount // XC_BUFS + 1))
            em.wait("pe", ("z", ph, 0, t))
            zi = zb_index[("z", ph, 0, t)]
            bank = zp[zi % NZB]
            if em.real:
                ins = nc.vector.tensor_tensor(
                    bank[:], bank[:], xc_sb[xcount % XC_BUFS][:], OP.add)
                if ph == "dec":
                    ins = nc.vector.tensor_tensor(bank[:], bank[:],
                                                  zpart_sb[:], OP.add)
                em.inc(ins, "dve", ("xcz", ph, t))
            else:
                em.inc(None, "dve", ("xcz", ph, t))

        def dve_body(e, real):
            em = Em(real, e)
            if em.real:
                for l in range(3):
                    nc.vector.memset(hT[l][:], 0)
                    nc.vector.memset(cst[l][:], 0)
                nc.vector.memset(zrT_sb[:], 0)
                ins = nc.vector.memset(partial_sb[:], 0)
                em.inc(ins, "dve", ("init",))
            else:
                em.inc(None, "dve", ("init",))
            for ph in ("enc", "dec"):
                if ph == "dec":
                    em.wait("pe", ("trCT",))
                    if em.real:
                        ins = nc.vector.tensor_copy(cT_sb[:], trF[:])
                        em.inc(ins, "dve", ("p2_cT",))
                    else:
                        em.inc(None, "dve", ("p2_cT",))
                    em.wait("act", ("p2_exp",))
                    if em.real:
                        nc.vector.tensor_tensor(zr_sb[:], els2_sb[:],
                                                eps_sb[:], OP.mult)
                        ins = nc.vector.tensor_tensor(zr_sb[:], zr_sb[:],
                                                      mean_sb[:], OP.add)
                        em.inc(ins, "dve", ("p2_zr",))
                    else:
                        em.inc(None, "dve", ("p2_zr",))
                    em.wait("pe", ("trZR",))
                    if em.real:
                        nc.vector.tensor_copy(zrT_sb[:, 0:BL],
                                              trF[:, 0:BL])
                        ins = nc.vector.tensor_copy(
                            zrT_sb[0:LAT - 128, BL:2 * BL],
                            trF[0:LAT - 128, BL:2 * BL])
                        em.inc(ins, "dve", ("p2_zrT",))
                    else:
                        em.inc(None, "dve", ("p2_zrT",))
                    em.wait("pe", ("zpart",))
                    zi = zb_index[("zpart",)]
                    if em.real:
                        ins = nc.vector.tensor_copy(zpart_sb[:],
                                                    zp[zi % NZB][:])
                        em.inc(ins, "dve", ("p2_zpart_copy",))
                    else:
                        em.inc(None, "dve", ("p2_zpart_copy",))
                    em.wait("pe", ("z", "enc", 2, T - 1))
                    em.wait("pe", ("trCT",))
                    if em.real:
                        for l in range(3):
                            nc.vector.memset(hT[l][:], 0)
                        nc.vector.memset(cst[0][:], 0)
                        nc.vector.memset(cst[1][:], 0)
                        ins = nc.vector.memset(cst[2][:], 0)
                        em.inc(ins, "dve", ("init2",))
                    else:
                        em.inc(None, "dve", ("init2",))
                for s in range(T + 3):
                    if 0 <= s - 3 < T:
                        dve_copy(em, ph, 2, s - 3)
                    if s < T:
                        dve_xcadd(em, ph, s, (0 if ph == "enc" else T) + s)
                        dve_cell(em, ph, 0, s)
                    if 0 <= s - 1 < T:
                        dve_cell(em, ph, 1, s - 1)
                    if s < T:
                        dve_copy(em, ph, 0, s)
                    if 0 <= s - 2 < T:
                        dve_cell(em, ph, 2, s - 2)
                    if 0 <= s - 1 < T:
                        dve_copy(em, ph, 1, s - 1)
            em.wait("act", ("precopy", T - 1))
            if em.real:
                nc.vector.tensor_tensor(ohm_sb[:], pre_buf[:], ohm_sb[:],
                                        OP.mult)
                ins = nc.vector.tensor_reduce(
                    out=partial_sb[:, 1:2], in_=ohm_sb[:],
                    axis=mybir.AxisListType.X, op=OP.add)
                em.inc(ins, "dve", ("f_picked",))
            else:
                em.inc(None, "dve", ("f_picked",))
            em.wait("act", ("f_exp",))
            if em.real:
                ins = nc.vector.tensor_reduce(
                    out=se_sb[:],
                    in_=pre_buf[:].rearrange("p (t v) -> p t v", v=V),
                    axis=mybir.AxisListType.X, op=OP.add)
                em.inc(ins, "dve", ("f_se",))
            else:
                em.inc(None, "dve", ("f_se",))
            em.wait("act", ("f_ln",))
            if em.real:
                nc.vector.tensor_tensor(se_sb[:], se_sb[:], maskf_sb[:],
                                        OP.mult)
                ins = nc.vector.tensor_reduce(
                    out=partial_sb[:, 0:1], in_=se_sb[:],
                    axis=mybir.AxisListType.X, op=OP.add)
                em.inc(ins, "dve", ("f_mlse",))
            else:
                em.inc(None, "dve", ("f_mlse",))
            em.wait("act", ("f_m2els",))
            if em.real:
                nc.vector.scalar_tensor_tensor(
                    out=els2_sb[:], in0=m2_sb[:], scalar=-1.0, in1=ls_sb[:],
                    op0=OP.mult, op1=OP.add)
                nc.vector.tensor_tensor(els_sb[:], els2_sb[:], els_sb[:],
                                        OP.subtract)
                nc.vector.tensor_scalar_add(els_sb[:], els_sb[:], 1.0)
                ins = nc.vector.tensor_reduce(
                    out=partial_sb[:, 2:3], in_=els_sb[:],
                    axis=mybir.AxisListType.X, op=OP.add)
                em.inc(ins, "dve", ("f_kl",))
            else:
                em.inc(None, "dve", ("f_kl",))

        def gp_body(e, real):
            em = Em(real, e)
            for ph in ("enc", "dec"):
                for s in range(T + 3):
                    for l, t in ((0, s), (1, s - 1), (2, s - 2)):
                        if not (0 <= t < T):
                            continue
                        em.wait("act", ("sg", ph, l, t))
                        if t > 0:
                            em.wait("dve", ("c2", ph, l, t - 1))
                        elif ph == "dec":
                            em.wait("dve", ("c2", "enc", l, T - 1))
                        if em.real:
                            ins = nc.gpsimd.tensor_tensor(
                                t2b[l][:], sg[l][:, 0:128],
                                sg[l][:, 384:512], OP.mult)
                            em.inc(ins, "gp", ("t2", ph, l, t))
                        else:
                            em.inc(None, "gp", ("t2", ph, l, t))

        def sync_body(e, real):
            em = Em(real, e)

            def dma(dst, src, sem, tag):
                if em.real:
                    e.dma_start(out=dst, in_=src).then_inc(SEMS[sem], 16)
                else:
                    tk.inc(sem, tag, 16)

            dma(iden_sb[:], iden_d[:], "w", "w_iden")
            dma(idenb_sb[:], idenb_d[:], "w", "w_idenb")
            dma(ones32_sb[:], ones32_d[:], "w", "w_ones32")
            dma(ohm_sb[:], ohm_d[:], "w", "w_ohm")
            dma(maskf_sb[:], maskf_d[:], "w", "w_maskf")
            dma(eps_sb[:], eps_d[:], "w", "w_eps")
            dma(wm_sb[:], wm_d[:].rearrange("p a b -> p (a b)"), "w", "w_wm")
            dma(ws_sb[:], ws_d[:].rearrange("p a b -> p (a b)"), "w", "w_ws")
            dma(wo_sb[:], wo_d[:].rearrange("p a b -> p (a b)"), "w", "w_wo")
            dma(dk0z_sb[:], dk0z_d[:].rearrange("p a b -> p (a b)"), "w",
                "w_dk0z")
            dma(rk0_sb[:], rk0e_d[:].rearrange("p a b -> p (a b)"), "w",
                "w_rk0e")
            dma(w1_sb[:], w1e_d[:].rearrange("p a b -> p (a b)"), "w",
                "w_w1e")
            dma(w2_sb[:], w2e_d[:].rearrange("p a b -> p (a b)"), "w",
                "w_w2e")
            for t in range(2 * T):
                ph = "enc" if t < T else "dec"
                tt = t if t < T else t - T
                if t >= XC_BUFS:
                    pt = t - XC_BUFS
                    em.wait("dve", ("xcz", "enc" if pt < T else "dec",
                                    pt if pt < T else pt - T))
                src = xc_enc_d if ph == "enc" else xc_dec_d
                dma(xc_sb[t % XC_BUFS][:], src[tt], f"xc{t % XC_BUFS}",
                    ("xc", t))
                if t == T - 1:
                    em.wait("pe", ("z", "enc", 2, T - 1))
                    dma(rk0_sb[:], rk0d_d[:].rearrange("p a b -> p (a b)"),
                        "w", "w_rk0d")
                    dma(w1_sb[:], w1d_d[:].rearrange("p a b -> p (a b)"),
                        "w", "w_w1d")
                    dma(w2_sb[:], w2d_d[:].rearrange("p a b -> p (a b)"),
                        "w", "w_w2d")
            em.wait("act", ("f_out",))
            dma(out_d[:], out_sb[:], "out", "out")
            if em.real:
                e.wait_ge(s_out, 16)

        pe_body(None, False)
        act_body(None, False)
        dve_body(None, False)
        gp_body(None, False)
        sync_body(None, False)
        tk.vals[("w", ("w_dec",))] = tk.vals[("w", "w_w2d")]
        tk.vals[("w", ("w_rk0e",))] = tk.vals[("w", "w_w2e")]
        tk.vals[("w", ("w_w1e",))] = tk.vals[("w", "w_w2e")]
        tk.vals[("w", ("w_w2e",))] = tk.vals[("w", "w_w2e")]

        @blk.tensor
        def _(e):
            pe_body(e, True)

        @blk.scalar
        def _(e):
            act_body(e, True)

        @blk.vector
        def _(e):
            dve_body(e, True)

        @blk.gpsimd
        def _(e):
            gp_body(e, True)

        @blk.sync
        def _(e):
            sync_body(e, True)

    return nc


def _zperm(M):
    D = M.shape[0]
    M4 = M.reshape(D, 4, 4, 128)[:, [0, 1, 3, 2], :, :]
    return np.ascontiguousarray(M4.transpose(0, 2, 1, 3).reshape(D, G4))


def _ktiles(w, kt):
    n = w.shape[1]
    return np.ascontiguousarray(
        w.reshape(kt, 128, n).transpose(1, 0, 2)).astype(bf16)


def _ktiles_pad(w, kt):
    r = kt * 128 - w.shape[0]
    if r:
        w = np.vstack([w, np.zeros((r, w.shape[1]), w.dtype)])
    return _ktiles(w, kt)


def _prep(inputs):
    X = np.asarray(inputs["X"]).astype(np.int64)[:, :T]
    Y = np.asarray(inputs["Y"]).astype(np.int64)[:, :T]
    C = np.asarray(inputs["C"]).astype(np.float32)
    L = np.asarray(inputs["L"]).astype(np.int64)
    eps = np.asarray(inputs["eps"]).astype(np.float32)
    f = lambda n: np.asarray(inputs[n]).astype(np.float32)
    emb_enc, emb_dec = f("emb_enc"), f("emb_dec")
    enc_k0, enc_b0 = f("enc_k0"), f("enc_b0")
    dec_k0, dec_b0 = f("dec_k0"), f("dec_b0")

    for nm in ("enc_b1", "enc_b2", "dec_b1", "dec_b2", "bm", "bs", "bo"):
        assert not np.any(np.asarray(inputs[nm])), f"nonzero bias {nm} unsupported"

    table_enc = emb_enc @ enc_k0[:LAT]
    table_dec = emb_dec @ dec_k0[LAT:2 * LAT]

    shared = {
        "rk0e": _ktiles(_zperm(f("enc_rk0")), KT),
        "w1e": _ktiles(_zperm(np.vstack([f("enc_k1"), f("enc_rk1")])), 2 * KT),
        "w2e": _ktiles(_zperm(np.vstack([f("enc_k2"), f("enc_rk2")])), 2 * KT),
        "rk0d": _ktiles(_zperm(f("dec_rk0")), KT),
        "w1d": _ktiles(_zperm(np.vstack([f("dec_k1"), f("dec_rk1")])), 2 * KT),
        "w2d": _ktiles(_zperm(np.vstack([f("dec_k2"), f("dec_rk2")])), 2 * KT),
        "dk0z": _ktiles_pad(_zperm(dec_k0[:LAT]), 2),
        "wm": _ktiles(f("Wm"), KT),
        "ws": _ktiles(f("Ws"), KT),
        "wo": _ktiles(f("Wo"), KT),
        "iden": np.eye(128, dtype=np.float32),
        "idenb": np.eye(128, dtype=np.float32).astype(bf16),
        "ones32": np.ones((BL, 1), np.float32),
    }

    in_maps = []
    tt = np.arange(T)
    vv = np.arange(V)
    for c in range(NCORES):
        sl = slice(c * BL, (c + 1) * BL)
        Xl, Yl, Cl, Ll, epsl = X[sl], Y[sl], C[sl], L[sl], eps[sl]
        cpart_e = Cl @ enc_k0[LAT:] + enc_b0
        cpart_d = Cl @ dec_k0[2 * LAT:] + dec_b0
        xce = table_enc[Xl] + cpart_e[:, None, :]
        xcd = table_dec[Xl] + cpart_d[:, None, :]

        def to_z(xc):
            xp = _zperm(xc.reshape(-1, G4)).reshape(BL, T, 4, 512)
            return np.ascontiguousarray(
                xp.transpose(1, 2, 0, 3).reshape(T, 128, 512)).astype(bf16)

        ohm = ((Yl[:, :, None] == vv) &
               (tt[None, :, None] < Ll[:, None, None]))
        ohm = ohm.reshape(BL, T * V).astype(bf16)
        maskf = (tt[None, :] < Ll[:, None]).astype(np.float32)
        m = dict(shared)
        m.update(xc_enc=to_z(xce), xc_dec=to_z(xcd), ohm=ohm, maskf=maskf,
                 eps=np.ascontiguousarray(epsl))
        in_maps.append(m)
    return in_maps


_BUILD_CACHE = {}


def kernel(**inputs):
    in_maps = _prep(inputs)
    if "nc" not in _BUILD_CACHE:
        _BUILD_CACHE["nc"] = _build()
    nc = _BUILD_CACHE["nc"]
    res = run_bass_kernel_spmd(nc, in_maps, list(range(NCORES)))
    mlse = picked = kl = 0.0
    for c in range(NCORES):
        o = np.asarray(res.results[c]["out"], np.float64).reshape(-1)
        mlse += o[0]
        picked += o[1]
        kl += o[2]
    recon = (mlse - picked) / (B * T)
    latent = -0.5 * (kl / (B * LAT))
    loss = recon + latent
    return (np.float32(loss), np.float32(recon), np.float32(latent))



# revision 16
# speedup vs baseline: 3.9607x; 3.9607x over previous
import sys as _sys

for _p in ("/opt/trn_rl_repo",):
    if _p not in _sys.path:
        _sys.path.insert(0, _p)

import os
from contextlib import ExitStack

import numpy as np
import ml_dtypes

import concourse.bass as bass
import concourse.mybir as mybir
from concourse.bass_utils import run_bass_kernel_spmd

bf16 = ml_dtypes.bfloat16
fp8 = ml_dtypes.float8_e4m3fn
F32 = mybir.dt.float32
BF = mybir.dt.bfloat16
F8 = mybir.dt.float8e4
DR = mybir.MatmulPerfMode.DoubleRow

B, V, LAT, U, P = 256, 42, 200, 512, 3
T = int(os.environ.get("KERNEL_T", "128"))
NCORES = 8
BL = B // NCORES
G4 = 4 * U
KT = U // 128
NGT = G4 // 128
AF = mybir.ActivationFunctionType
OP = mybir.AluOpType

XC_BUFS = 4
NZB = 3
PHASES = tuple(os.environ.get("KERNEL_PHASES", "enc,dec").split(","))


class Tracker:

    def __init__(self):
        self.vals = {}
        self.counts = {}

    def inc(self, sem, tag, n=1):
        c = self.counts.get(sem, 0) + n
        self.counts[sem] = c
        key = (sem, tag)
        assert key not in self.vals, f"dup inc tag {key}"
        self.vals[key] = c
        return c

    def val(self, sem, tag):
        return self.vals[(sem, tag)]


def _build():
    nc = bass.Bass()

    dp = lambda n, s, d, o=False: nc.declare_dram_parameter(n, list(s), d, isOutput=o)
    xc_enc_d = dp("xc_enc", [T, 128, NGT * BL], BF)
    xc_dec_d = dp("xc_dec", [T, 128, NGT * BL], BF)
    rk0e_d = dp("rk0e", [128, KT, G4], F8)
    w1e_d = dp("w1e", [128, 2 * KT, G4], F8)
    w2e_d = dp("w2e", [128, 2 * KT, G4], F8)
    rk0d_d = dp("rk0d", [128, KT, G4], F8)
    w1d_d = dp("w1d", [128, 2 * KT, G4], F8)
    w2d_d = dp("w2d", [128, 2 * KT, G4], F8)
    dk0z_d = dp("dk0z", [128, 2, G4], F8)
    wm_d = dp("wm", [128, KT, LAT], BF)
    ws_d = dp("ws", [128, KT, LAT], BF)
    wo_d = dp("wo", [128, KT, V], BF)
    ohm_d = dp("ohm", [BL, T * V], BF)
    maskf_d = dp("maskf", [BL, T], F32)
    eps_d = dp("eps", [BL, LAT], F32)
    iden_d = dp("iden", [128, 128], F32)
    idenb_d = dp("idenb", [128, 128], BF)
    ones32_d = dp("ones32", [BL, 1], F32)
    out_d = dp("out", [1, 4], F32, o=True)

    tk = Tracker()

    with ExitStack() as ctx:
        _nn = [0]

        def sbt(*s):
            _nn[0] += 1
            return ctx.enter_context(
                nc.sbuf_tensor(f"sb{_nn[0]}", list(s[:-1]), s[-1]))

        def pst(*s):
            _nn[0] += 1
            return ctx.enter_context(
                nc.psum_tensor(f"ps{_nn[0]}", list(s[:-1]), s[-1]))

        rk0_sb = sbt(128, KT * G4, F8)
        w1_sb = sbt(128, 2 * KT * G4, F8)
        w2_sb = sbt(128, 2 * KT * G4, F8)
        dk0z_sb = sbt(128, 2 * G4, F8)
        wm_sb = sbt(128, KT * LAT, BF)
        ws_sb = sbt(128, KT * LAT, BF)
        wo_sb = sbt(128, KT * V, BF)
        iden_sb = sbt(128, 128, F32)
        idenb_sb = sbt(128, 128, BF)
        ones32_sb = sbt(BL, 1, F32)
        xc_sb = [sbt(128, NGT * BL, BF) for _ in range(XC_BUFS)]
        hT = [sbt(128, KT * BL, F8) for _ in range(3)]
        cst = [sbt(128, KT * BL, F32) for _ in range(3)]
        sg = [sbt(128, 512, F32) for _ in range(3)]
        t1b = [sbt(128, 128, F32) for _ in range(3)]
        t2b = [sbt(128, 128, F32) for _ in range(3)]
        tcb = [sbt(128, 128, F32) for _ in range(3)]
        cTb_sb = sbt(128, KT * BL, BF)
        zrT_sb = sbt(128, 2 * BL, F8)
        zpart_sb = sbt(128, 512, BF)
        ohm_sb = sbt(BL, T * V, BF)
        maskf_sb = sbt(BL, T, F32)
        eps_sb = sbt(BL, LAT, F32)
        mean_sb = sbt(BL, LAT, F32)
        ls_sb = sbt(BL, LAT, F32)
        els2_sb = sbt(BL, LAT, F32)
        zr_sb = sbt(BL, LAT, F32)
        pre_buf = sbt(BL, T * V, F32)
        se_sb = sbt(BL, T, F32)
        m2_sb = sbt(BL, LAT, F32)
        els_sb = sbt(BL, LAT, F32)
        partial_sb = sbt(BL, 4, F32)
        out_sb = sbt(1, 4, F32)

        zp = [pst(128, 512, F32) for _ in range(NZB)]
        preP = [pst(BL, 512, F32), pst(BL, 512, F32)]
        trF = pst(128, 128, F32)

        s_pe = ctx.enter_context(nc.semaphore("s_pe"))
        s_act = ctx.enter_context(nc.semaphore("s_act"))
        s_dve = ctx.enter_context(nc.semaphore("s_dve"))
        s_w = ctx.enter_context(nc.semaphore("s_w"))
        s_out = ctx.enter_context(nc.semaphore("s_out"))
        s_xcb = [ctx.enter_context(nc.semaphore(f"s_xc{i}"))
                 for i in range(XC_BUFS)]
        blk = ctx.enter_context(nc.Block())

        SEMS = {"pe": s_pe, "act": s_act, "dve": s_dve,
                "w": s_w, "out": s_out}
        for i in range(XC_BUFS):
            SEMS[f"xc{i}"] = s_xcb[i]

        class Em:
            def __init__(self, real, eng=None):
                self.real = real
                self.eng = eng

            def wait(self, sem, tag):
                if self.real:
                    self.eng.wait_ge(SEMS[sem], tk.val(sem, tag))

            def wait_val(self, sem, v):
                if self.real:
                    self.eng.wait_ge(SEMS[sem], v)

            def inc(self, ins, sem, tag, n=1):
                if not self.real:
                    tk.inc(sem, tag, n)
                else:
                    ins.then_inc(SEMS[sem], n)

        zburst_list = []
        for ph in PHASES:
            if ph == "dec":
                zburst_list.append(("zpart",))
            for s in range(T + 3):
                for l, t in ((0, s), (1, s - 1), (2, s - 2)):
                    if 0 <= t < T:
                        zburst_list.append(("z", ph, l, t))
        zb_index = {e: i for i, e in enumerate(zburst_list)}

        def zburst_reader(e):
            if e[0] == "zpart":
                return ("dve", ("p2_zpart_copy",))
            _, ph, l, t = e
            return ("act", ("sg", ph, l, t))

        def wsb_for(l):
            return (rk0_sb, w1_sb, w2_sb)[l]

        def pe_zburst(em, ph, l, t):
            e = ("z", ph, l, t)
            zi = zb_index[e]
            bank = zp[zi % NZB]
            if zi >= NZB:
                em.wait(*zburst_reader(zburst_list[zi - NZB]))
            if t == 0:
                if ph == "enc":
                    em.wait("w", ("w_rk0e",) if l == 0 else
                            (("w_w1e",) if l == 1 else ("w_w2e",)))
                else:
                    em.wait("w", ("w_dec",))
            if t == 0:
                em.wait("dve", ("init",) if ph == "enc" else ("init2",))
            else:
                em.wait("dve", ("h2", ph, l, t - 1))
            if l > 0:
                em.wait("dve", ("h2", ph, l - 1, t))
            wsb = wsb_for(l)
            npair = (KT if l == 0 else 2 * KT) // 2

            def mov(j, l=l):
                if l == 0 or j < KT // 2:
                    src = hT[0] if l == 0 else hT[l - 1]
                    sl = src[:, 2 * j * BL:(2 * j + 2) * BL]
                else:
                    jj = j - KT // 2
                    sl = hT[l][:, 2 * jj * BL:(2 * jj + 2) * BL]
                return sl.rearrange("p (two c) -> p two c", two=2)

            def wpair(j, g):
                sl = wsb[:, (j * NGT + g) * 256:(j * NGT + g + 1) * 256]
                return sl.rearrange("p (two c) -> p two c", two=2)

            if em.real:
                if l == 0:
                    xcount = (0 if ph == "enc" else T) + t
                    em.wait_val(f"xc{xcount % XC_BUFS}",
                                16 * (xcount // XC_BUFS + 1))
                    nc.tensor.matmul(
                        bank[:], idenb_sb[:], xc_sb[xcount % XC_BUFS][:],
                        start=True, stop=False, skip_group_check=True)
                    if ph == "dec":
                        if t == 0:
                            em.wait("dve", ("p2_zpart_copy",))
                        nc.tensor.matmul(
                            bank[:], idenb_sb[:], zpart_sb[:],
                            start=False, stop=False, skip_group_check=True)
                    for g in range(NGT):
                        for j in range(npair):
                            ins = nc.tensor.matmul(
                                bank[:, g * BL:(g + 1) * BL],
                                wpair(j, g), mov(j),
                                start=False,
                                stop=(g == NGT - 1 and j == npair - 1),
                                perf_mode=DR, skip_group_check=True,
                            )
                else:
                    for g in range(NGT):
                        for j in range(npair):
                            ins = nc.tensor.matmul(
                                bank[:, g * BL:(g + 1) * BL],
                                wpair(j, g), mov(j),
                                start=(j == 0),
                                stop=(j == npair - 1),
                                perf_mode=DR, skip_group_check=True,
                            )
                em.inc(ins, "pe", e)
            else:
                em.inc(None, "pe", e)

        def pe_proj(em, t):
            em.wait("dve", ("h2", "dec", 2, t))
            if t >= 2:
                em.wait("act", ("precopy", t - 2))
            elif t == 0:
                em.wait("act", ("p2_mean_sb",))
            else:
                em.wait("act", ("p2_ls_sb",))
            pp = preP[t % 2]
            if em.real:
                for k in range(KT):
                    ins = nc.tensor.matmul(
                        pp[:, 0:V], hT[2][:, k * BL:(k + 1) * BL],
                        wo_sb[:, k * V:(k + 1) * V],
                        start=(k == 0), stop=(k == KT - 1))
                em.inc(ins, "pe", ("proj", t))
            else:
                em.inc(None, "pe", ("proj", t))

        def pe_body(e, real):
            em = Em(real, e)
            for ph in PHASES:
                if ph == "dec":
                    em.wait("act", ("p2_cTb",))
                    for dst, wsb2, tg in ((preP[0], wm_sb, "p2_mm_mean"),
                                          (preP[1], ws_sb, "p2_mm_ls")):
                        if em.real:
                            for k in range(KT):
                                ins = nc.tensor.matmul(
                                    dst[:, 0:LAT],
                                    cTb_sb[:, k * BL:(k + 1) * BL],
                                    wsb2[:, k * LAT:(k + 1) * LAT],
                                    start=(k == 0), stop=(k == KT - 1))
                            em.inc(ins, "pe", (tg,))
                        else:
                            em.inc(None, "pe", (tg,))
                    em.wait("dve", ("p2_zr",))
                    if em.real:
                        nc.tensor.transpose(trF[:, 0:BL], zr_sb[:, 0:128],
                                            iden_sb[0:BL, 0:BL])
                        ins = nc.tensor.transpose(
                            trF[0:LAT - 128, BL:2 * BL], zr_sb[:, 128:LAT],
                            iden_sb[0:BL, 0:BL])
                        em.inc(ins, "pe", ("trZR",))
                    else:
                        em.inc(None, "pe", ("trZR",))
                    em.wait("dve", ("p2_zrT",))
                    zi = zb_index[("zpart",)]
                    em.wait(*zburst_reader(zburst_list[zi - NZB]))
                    bank = zp[zi % NZB]
                    if em.real:
                        zrm = zrT_sb[:].rearrange("p (two c) -> p two c",
                                                  two=2)
                        for g in range(NGT):
                            ins = nc.tensor.matmul(
                                bank[:, g * BL:(g + 1) * BL],
                                dk0z_sb[:, g * 256:(g + 1) * 256].rearrange(
                                    "p (two c) -> p two c", two=2),
                                zrm,
                                start=True, stop=True,
                                perf_mode=DR, skip_group_check=True,
                            )
                        em.inc(ins, "pe", ("zpart",))
                    else:
                        em.inc(None, "pe", ("zpart",))

                for s in range(T + 3):
                    if s < T:
                        pe_zburst(em, ph, 0, s)
                    if 0 <= s - 1 < T:
                        pe_zburst(em, ph, 1, s - 1)
                    if 0 <= s - 2 < T:
                        pe_zburst(em, ph, 2, s - 2)
                    if ph == "dec" and 0 <= s - 3 < T:
                        pe_proj(em, s - 3)

            if "dec" not in PHASES:
                return
            em.wait("dve", ("f_kl",))
            if em.real:
                ins = nc.tensor.matmul(preP[0][0:1, 300:304], ones32_sb[:],
                                       partial_sb[:], start=True, stop=True)
                em.inc(ins, "pe", ("f_red",))
            else:
                em.inc(None, "pe", ("f_red",))

        def act_sig(em, ph, l, t):
            em.wait("pe", ("z", ph, l, t))
            if t > 0:
                em.wait("dve", ("h2", ph, l, t - 1))
            elif ph == "dec":
                em.wait("dve", ("h2", "enc", l, T - 1))
            zi = zb_index[("z", ph, l, t)]
            bank = zp[zi % NZB]
            if em.real:
                nc.scalar.activation(sg[l][:, 0:384], bank[:, 0:384],
                                     AF.Sigmoid)
                ins = nc.scalar.activation(sg[l][:, 384:512],
                                           bank[:, 384:512], AF.Tanh)
                em.inc(ins, "act", ("sg", ph, l, t))
            else:
                em.inc(None, "act", ("sg", ph, l, t))

        def act_tc(em, ph, l, t):
            em.wait("dve", ("c2", ph, l, t))
            if em.real:
                ins = nc.scalar.activation(tcb[l][:], cst[l][:], AF.Tanh)
                em.inc(ins, "act", ("tc", ph, l, t))
            else:
                em.inc(None, "act", ("tc", ph, l, t))

        def act_body(e, real):
            em = Em(real, e)
            _final = "dec" in PHASES
            for ph in PHASES:
                if ph == "dec":
                    em.wait("dve", ("c2", "enc", 2, T - 1))
                    if em.real:
                        ins = nc.scalar.copy(cTb_sb[:], cst[2][:])
                        em.inc(ins, "act", ("p2_cTb",))
                    else:
                        em.inc(None, "act", ("p2_cTb",))
                    em.wait("pe", ("p2_mm_mean",))
                    if em.real:
                        ins = nc.scalar.copy(mean_sb[:], preP[0][:, 0:LAT])
                        em.inc(ins, "act", ("p2_mean_sb",))
                    else:
                        em.inc(None, "act", ("p2_mean_sb",))
                    em.wait("pe", ("p2_mm_ls",))
                    if em.real:
                        ins = nc.scalar.copy(ls_sb[:], preP[1][:, 0:LAT])
                        em.inc(ins, "act", ("p2_ls_sb",))
                    else:
                        em.inc(None, "act", ("p2_ls_sb",))
                    if em.real:
                        ins = nc.scalar.activation(els2_sb[:], ls_sb[:],
                                                   AF.Exp, scale=0.5)
                        em.inc(ins, "act", ("p2_exp",))
                    else:
                        em.inc(None, "act", ("p2_exp",))
                for s in range(T + 3):
                    if 0 <= s < T:
                        act_sig(em, ph, 0, s)
                    if 0 <= s - 3 < T:
                        act_tc(em, ph, 2, s - 3)
                    if 0 <= s < T:
                        act_tc(em, ph, 0, s)
                    if 0 <= s - 1 < T:
                        act_sig(em, ph, 1, s - 1)
                        act_tc(em, ph, 1, s - 1)
                    if 0 <= s - 2 < T:
                        act_sig(em, ph, 2, s - 2)
                    if ph == "dec" and 0 <= s - 3 < T:
                        tt = s - 3
                        em.wait("pe", ("proj", tt))
                        if em.real:
                            ins = nc.scalar.copy(
                                pre_buf[:, tt * V:(tt + 1) * V],
                                preP[tt % 2][:, 0:V])
                            em.inc(ins, "act", ("precopy", tt))
                        else:
                            em.inc(None, "act", ("precopy", tt))
            if not _final:
                return
            em.wait("dve", ("f_picked",))
            if em.real:
                ins = nc.scalar.activation(pre_buf[:], pre_buf[:], AF.Exp)
                em.inc(ins, "act", ("f_exp",))
            else:
                em.inc(None, "act", ("f_exp",))
            em.wait("dve", ("f_se",))
            if em.real:
                ins = nc.scalar.activation(se_sb[:], se_sb[:], AF.Ln)
                em.inc(ins, "act", ("f_ln",))
            else:
                em.inc(None, "act", ("f_ln",))
            if em.real:
                nc.scalar.activation(m2_sb[:], mean_sb[:], AF.Square)
                ins = nc.scalar.activation(els_sb[:], ls_sb[:], AF.Exp)
                em.inc(ins, "act", ("f_m2els",))
            else:
                em.inc(None, "act", ("f_m2els",))
            em.wait("pe", ("f_red",))
            if em.real:
                ins = nc.scalar.copy(out_sb[:], preP[0][0:1, 300:304])
                em.inc(ins, "act", ("f_out",))
            else:
                em.inc(None, "act", ("f_out",))

        def dve_c2(em, ph, l, t):
            em.wait("act", ("sg", ph, l, t))
            if em.real:
                nc.vector.tensor_tensor(t1b[l][:], sg[l][:, 128:256],
                                        cst[l][:], OP.mult)
                nc.vector.tensor_tensor(t2b[l][:], sg[l][:, 0:128],
                                        sg[l][:, 384:512], OP.mult)
            if t > 0:
                em.wait("act", ("tc", ph, l, t - 1))
            elif ph == "dec":
                em.wait("act", ("tc", "enc", l, T - 1))
            if em.real:
                ins = nc.vector.tensor_tensor(cst[l][:], t1b[l][:],
                                              t2b[l][:], OP.add)
                em.inc(ins, "dve", ("c2", ph, l, t))
            else:
                em.inc(None, "dve", ("c2", ph, l, t))

        def dve_h2(em, ph, l, t):
            em.wait("act", ("tc", ph, l, t))
            if l < 2:
                if t > 0:
                    em.wait("pe", ("z", ph, l + 1, t - 1))
            elif ph == "dec" and t > 0:
                em.wait("pe", ("proj", t - 1))
            if em.real:
                ins = nc.vector.tensor_tensor(hT[l][:], sg[l][:, 256:384],
                                              tcb[l][:], OP.mult)
                em.inc(ins, "dve", ("h2", ph, l, t))
            else:
                em.inc(None, "dve", ("h2", ph, l, t))

        def dve_body(e, real):
            em = Em(real, e)
            if em.real:
                for l in range(3):
                    nc.vector.memset(hT[l][:], 0)
                    nc.vector.memset(cst[l][:], 0)
                nc.vector.memset(zrT_sb[:], 0)
                ins = nc.vector.memset(partial_sb[:], 0)
                em.inc(ins, "dve", ("init",))
            else:
                em.inc(None, "dve", ("init",))
            for ph in PHASES:
                if ph == "dec":
                    em.wait("act", ("p2_exp",))
                    if em.real:
                        nc.vector.tensor_tensor(zr_sb[:], els2_sb[:],
                                                eps_sb[:], OP.mult)
                        ins = nc.vector.tensor_tensor(zr_sb[:], zr_sb[:],
                                                      mean_sb[:], OP.add)
                        em.inc(ins, "dve", ("p2_zr",))
                    else:
                        em.inc(None, "dve", ("p2_zr",))
                    em.wait("pe", ("trZR",))
                    if em.real:
                        nc.vector.tensor_copy(zrT_sb[:, 0:BL],
                                              trF[:, 0:BL])
                        ins = nc.vector.tensor_copy(
                            zrT_sb[0:LAT - 128, BL:2 * BL],
                            trF[0:LAT - 128, BL:2 * BL])
                        em.inc(ins, "dve", ("p2_zrT",))
                    else:
                        em.inc(None, "dve", ("p2_zrT",))
                    em.wait("pe", ("zpart",))
                    zi = zb_index[("zpart",)]
                    if em.real:
                        ins = nc.vector.tensor_copy(zpart_sb[:],
                                                    zp[zi % NZB][:])
                        em.inc(ins, "dve", ("p2_zpart_copy",))
                    else:
                        em.inc(None, "dve", ("p2_zpart_copy",))
                    em.wait("pe", ("z", "enc", 2, T - 1))
                    em.wait("act", ("p2_cTb",))
                    if em.real:
                        for l in range(3):
                            nc.vector.memset(hT[l][:], 0)
                        nc.vector.memset(cst[0][:], 0)
                        nc.vector.memset(cst[1][:], 0)
                        ins = nc.vector.memset(cst[2][:], 0)
                        em.inc(ins, "dve", ("init2",))
                    else:
                        em.inc(None, "dve", ("init2",))
                for s in range(T + 3):
                    if s < T:
                        dve_c2(em, ph, 0, s)
                    if 0 <= s - 3 < T:
                        dve_h2(em, ph, 2, s - 3)
                    if s < T:
                        dve_h2(em, ph, 0, s)
                    if 0 <= s - 1 < T:
                        dve_c2(em, ph, 1, s - 1)
                        dve_h2(em, ph, 1, s - 1)
                    if 0 <= s - 2 < T:
                        dve_c2(em, ph, 2, s - 2)
            if "dec" not in PHASES:
                return
            em.wait("act", ("precopy", T - 1))
            if em.real:
                nc.vector.tensor_tensor(ohm_sb[:], pre_buf[:], ohm_sb[:],
                                        OP.mult)
                ins = nc.vector.tensor_reduce(
                    out=partial_sb[:, 1:2], in_=ohm_sb[:],
                    axis=mybir.AxisListType.X, op=OP.add)
                em.inc(ins, "dve", ("f_picked",))
            else:
                em.inc(None, "dve", ("f_picked",))
            em.wait("act", ("f_exp",))
            if em.real:
                ins = nc.vector.tensor_reduce(
                    out=se_sb[:],
                    in_=pre_buf[:].rearrange("p (t v) -> p t v", v=V),
                    axis=mybir.AxisListType.X, op=OP.add)
                em.inc(ins, "dve", ("f_se",))
            else:
                em.inc(None, "dve", ("f_se",))
            em.wait("act", ("f_ln",))
            if em.real:
                nc.vector.tensor_tensor(se_sb[:], se_sb[:], maskf_sb[:],
                                        OP.mult)
                ins = nc.vector.tensor_reduce(
                    out=partial_sb[:, 0:1], in_=se_sb[:],
                    axis=mybir.AxisListType.X, op=OP.add)
                em.inc(ins, "dve", ("f_mlse",))
            else:
                em.inc(None, "dve", ("f_mlse",))
            em.wait("act", ("f_m2els",))
            if em.real:
                nc.vector.scalar_tensor_tensor(
                    out=els2_sb[:], in0=m2_sb[:], scalar=-1.0, in1=ls_sb[:],
                    op0=OP.mult, op1=OP.add)
                nc.vector.tensor_tensor(els_sb[:], els2_sb[:], els_sb[:],
                                        OP.subtract)
                nc.vector.tensor_scalar_add(els_sb[:], els_sb[:], 1.0)
                ins = nc.vector.tensor_reduce(
                    out=partial_sb[:, 2:3], in_=els_sb[:],
                    axis=mybir.AxisListType.X, op=OP.add)
                em.inc(ins, "dve", ("f_kl",))
            else:
                em.inc(None, "dve", ("f_kl",))

        def sync_body(e, real):
            em = Em(real, e)

            def dma(dst, src, sem, tag):
                if em.real:
                    e.dma_start(out=dst, in_=src).then_inc(SEMS[sem], 16)
                else:
                    tk.inc(sem, tag, 16)

            dma(iden_sb[:], iden_d[:], "w", "w_iden")
            dma(idenb_sb[:], idenb_d[:], "w", "w_idenb")
            dma(ones32_sb[:], ones32_d[:], "w", "w_ones32")
            dma(ohm_sb[:], ohm_d[:], "w", "w_ohm")
            dma(maskf_sb[:], maskf_d[:], "w", "w_maskf")
            dma(eps_sb[:], eps_d[:], "w", "w_eps")
            dma(wm_sb[:], wm_d[:].rearrange("p a b -> p (a b)"), "w", "w_wm")
            dma(ws_sb[:], ws_d[:].rearrange("p a b -> p (a b)"), "w", "w_ws")
            dma(wo_sb[:], wo_d[:].rearrange("p a b -> p (a b)"), "w", "w_wo")
            dma(dk0z_sb[:], dk0z_d[:].rearrange("p a b -> p (a b)"), "w",
                "w_dk0z")
            dma(rk0_sb[:], rk0e_d[:].rearrange("p a b -> p (a b)"), "w",
                "w_rk0e")
            dma(w1_sb[:], w1e_d[:].rearrange("p a b -> p (a b)"), "w",
                "w_w1e")
            dma(w2_sb[:], w2e_d[:].rearrange("p a b -> p (a b)"), "w",
                "w_w2e")
            for t in range(2 * T if "dec" in PHASES else T):
                ph = "enc" if t < T else "dec"
                tt = t if t < T else t - T
                if t >= XC_BUFS:
                    pt = t - XC_BUFS
                    em.wait("pe", ("z", "enc" if pt < T else "dec", 0,
                                   pt if pt < T else pt - T))
                src = xc_enc_d if ph == "enc" else xc_dec_d
                dma(xc_sb[t % XC_BUFS][:], src[tt], f"xc{t % XC_BUFS}",
                    ("xc", t))
                if t == T - 1:
                    em.wait("pe", ("z", "enc", 2, T - 1))
                    dma(rk0_sb[:], rk0d_d[:].rearrange("p a b -> p (a b)"),
                        "w", "w_rk0d")
                    dma(w1_sb[:], w1d_d[:].rearrange("p a b -> p (a b)"),
                        "w", "w_w1d")
                    dma(w2_sb[:], w2d_d[:].rearrange("p a b -> p (a b)"),
                        "w", "w_w2d")
            if "dec" in PHASES:
                em.wait("act", ("f_out",))
                dma(out_d[:], out_sb[:], "out", "out")
                if em.real:
                    e.wait_ge(s_out, 16)

        pe_body(None, False)
        act_body(None, False)
        dve_body(None, False)
        sync_body(None, False)
        if "dec" in PHASES:
            tk.vals[("w", ("w_dec",))] = tk.vals[("w", "w_w2d")]
        tk.vals[("w", ("w_rk0e",))] = tk.vals[("w", "w_w2e")]
        tk.vals[("w", ("w_w1e",))] = tk.vals[("w", "w_w2e")]
        tk.vals[("w", ("w_w2e",))] = tk.vals[("w", "w_w2e")]

        @blk.tensor
        def _(e):
            pe_body(e, True)

        @blk.scalar
        def _(e):
            act_body(e, True)

        @blk.vector
        def _(e):
            dve_body(e, True)

        @blk.sync
        def _(e):
            sync_body(e, True)

    return nc


def _zpermT(M):
    D = M.shape[0]
    M4 = M.reshape(D, 4, 4, 128)[:, [0, 1, 3, 2], :, :]
    return np.ascontiguousarray(M4.reshape(D, G4))


def _ktiles(w, kt):
    n = w.shape[1]
    return np.ascontiguousarray(
        w.reshape(kt, 128, n).transpose(1, 0, 2)).astype(bf16)


def _kpairs(w, kt):
    npair = kt // 2
    w5 = w.reshape(npair, 2, 128, NGT, 128)
    out = np.ascontiguousarray(w5.transpose(2, 0, 3, 1, 4)).reshape(
        128, kt, G4)
    return out.astype(fp8)


def _kpairs_pad(w, kt):
    r = kt * 128 - w.shape[0]
    if r:
        w = np.vstack([w, np.zeros((r, w.shape[1]), w.dtype)])
    return _kpairs(w, kt)


def _prep(inputs):
    X = np.asarray(inputs["X"]).astype(np.int64)[:, :T]
    Y = np.asarray(inputs["Y"]).astype(np.int64)[:, :T]
    C = np.asarray(inputs["C"]).astype(np.float32)
    L = np.asarray(inputs["L"]).astype(np.int64)
    eps = np.asarray(inputs["eps"]).astype(np.float32)
    f = lambda n: np.asarray(inputs[n]).astype(np.float32)
    emb_enc, emb_dec = f("emb_enc"), f("emb_dec")
    enc_k0, enc_b0 = f("enc_k0"), f("enc_b0")
    dec_k0, dec_b0 = f("dec_k0"), f("dec_b0")

    for nm in ("enc_b1", "enc_b2", "dec_b1", "dec_b2", "bm", "bs", "bo"):
        assert not np.any(np.asarray(inputs[nm])), f"nonzero bias {nm} unsupported"

    table_enc = emb_enc @ enc_k0[:LAT]
    table_dec = emb_dec @ dec_k0[LAT:2 * LAT]

    shared = {
        "rk0e": _kpairs(_zpermT(f("enc_rk0")), KT),
        "w1e": _kpairs(_zpermT(np.vstack([f("enc_k1"), f("enc_rk1")])), 2 * KT),
        "w2e": _kpairs(_zpermT(np.vstack([f("enc_k2"), f("enc_rk2")])), 2 * KT),
        "rk0d": _kpairs(_zpermT(f("dec_rk0")), KT),
        "w1d": _kpairs(_zpermT(np.vstack([f("dec_k1"), f("dec_rk1")])), 2 * KT),
        "w2d": _kpairs(_zpermT(np.vstack([f("dec_k2"), f("dec_rk2")])), 2 * KT),
        "dk0z": _kpairs_pad(_zpermT(dec_k0[:LAT]), 2),
        "wm": _ktiles(f("Wm"), KT),
        "ws": _ktiles(f("Ws"), KT),
        "wo": _ktiles(f("Wo"), KT),
        "iden": np.eye(128, dtype=np.float32),
        "idenb": np.eye(128, dtype=np.float32).astype(bf16),
        "ones32": np.ones((BL, 1), np.float32),
    }

    in_maps = []
    tt = np.arange(T)
    vv = np.arange(V)
    for c in range(NCORES):
        sl = slice(c * BL, (c + 1) * BL)
        Xl, Yl, Cl, Ll, epsl = X[sl], Y[sl], C[sl], L[sl], eps[sl]
        cpart_e = Cl @ enc_k0[LAT:] + enc_b0
        cpart_d = Cl @ dec_k0[2 * LAT:] + dec_b0
        xce = table_enc[Xl] + cpart_e[:, None, :]
        xcd = table_dec[Xl] + cpart_d[:, None, :]

        def to_zT(xc):
            xp = _zpermT(xc.reshape(-1, G4)).reshape(BL, T, NGT, 128)
            return np.ascontiguousarray(
                xp.transpose(1, 3, 2, 0).reshape(T, 128, NGT * BL)).astype(bf16)

        ohm = ((Yl[:, :, None] == vv) &
               (tt[None, :, None] < Ll[:, None, None]))
        ohm = ohm.reshape(BL, T * V).astype(bf16)
        maskf = (tt[None, :] < Ll[:, None]).astype(np.float32)
        m = dict(shared)
        m.update(xc_enc=to_zT(xce), xc_dec=to_zT(xcd), ohm=ohm, maskf=maskf,
                 eps=np.ascontiguousarray(epsl))
        in_maps.append(m)
    return in_maps


_BUILD_CACHE = {}


def kernel(**inputs):
    in_maps = _prep(inputs)
    if "nc" not in _BUILD_CACHE:
        _BUILD_CACHE["nc"] = _build()
    nc = _BUILD_CACHE["nc"]
    res = run_bass_kernel_spmd(nc, in_maps, list(range(NCORES)))
    mlse = picked = kl = 0.0
    for c in range(NCORES):
        o = np.asarray(res.results[c]["out"], np.float64).reshape(-1)
        mlse += o[0]
        picked += o[1]
        kl += o[2]
    recon = (mlse - picked) / (B * T)
    latent = -0.5 * (kl / (B * LAT))
    loss = recon + latent
    return (np.float32(loss), np.float32(recon), np.float32(latent))
